# revision 8
# baseline (speedup 1.0000x reference)
"""Trainium2 Bass kernel for nn_AttentionBlock (B=8, C=512, H=W=32, heads=8, groups=32).

Sharding: data-parallel over batch B across the 8 NeuronCores (1 batch element
per core, no collectives). Each core computes, for its X slice [512, 1024]:

    GroupNorm -> qkv 1x1 conv -> 8-head attention (S=1024, hd=64) -> proj -> +residual

Key layout choices (all matmuls are float32r = tf32-like, psum accumulation fp32):
  - X, Xn, Q, K channel-major [C, S]; V produced pre-transposed as [S, C_v] by
    swapping matmul operands, so attention needs no explicit transposes.
  - scores^T[k, q] per head via K=64 matmuls, two heads packed in the PE array
    with row-tiling (heads 2p/2p+1 live in partitions 0-63/64-127 of chunk p).
  - exp on the scalar engine directly from PSUM with the 1/sqrt(hd) scale fused.
  - attn@V and the softmax denominator (ones-vector matmul) are col-tiled.
  - denominators are reciprocal'd with the fast custom-DVE op after a DMA
    round-trip that spreads them over 128 partitions; the normalize multiply is
    fused into the PSUM eviction of attn@V.
  - proj bias + residual fused into one scalar_tensor_tensor eviction per chunk.
"""
import numpy as np
from contextlib import ExitStack

import concourse.bacc as bacc
import concourse.bass as bass
import concourse.tile as tile
from concourse import mybir
from concourse.bass_utils import run_bass_kernel_spmd

F32 = mybir.dt.float32
F32R = mybir.dt.float32r
AF = mybir.ActivationFunctionType

B, C, H, W = 8, 512, 32, 32
S = H * W            # 1024
NH = 8               # heads
HD = C // NH         # 64
NG = 32              # groups
GS = C // NG         # 16 channels per group
EPS = 1e-5
NCC = C // 128       # 4 channel chunks
NSC = S // 128       # 8 sequence chunks of 128
NQ = S // 512        # 2 q-chunks of 512
SCALE = HD ** -0.5   # 0.125


def build_nc():
    nc = bacc.Bacc("TRN2", target_bir_lowering=False, debug=False)

    # ---- DRAM parameters (per-core). f32r tensors carry plain fp32 bits; the
    # PE rounds on read. Order of declaration = binding order.
    x_d = nc.declare_dram_parameter("x", [C, S], F32R, isOutput=False)
    qkvw_d = nc.declare_dram_parameter("qkv_wT", [C, 3 * C], F32R, isOutput=False)
    projw_d = nc.declare_dram_parameter("proj_wT", [C, C], F32R, isOutput=False)
    gsum_d = nc.declare_dram_parameter("gsum", [C, NG], F32R, isOutput=False)
    gexp_d = nc.declare_dram_parameter("gexpT", [NG, C], F32R, isOutput=False)
    w4_d = nc.declare_dram_parameter("norm_w4", [128, NCC], F32, isOutput=False)
    b4_d = nc.declare_dram_parameter("norm_b4", [128, NCC], F32, isOutput=False)
    qb_d = nc.declare_dram_parameter("qkv_b12", [128, 12], F32, isOutput=False)
    vb_d = nc.declare_dram_parameter("vb_bcast", [128, C], F32, isOutput=False)
    pb_d = nc.declare_dram_parameter("proj_b4", [128, NCC], F32, isOutput=False)
    ones_d = nc.declare_dram_parameter("ones128", [128, 1], F32R, isOutput=False)
    y_d = nc.declare_dram_parameter("y", [C, S], F32, isOutput=True)

    # DRAM scratch for the softmax-denominator reciprocal round-trip.
    # layout [pair][qn][head-in-pair][q512]
    dens_d = nc.dram_tensor("dens_scratch", [NH // 2, NQ, 2, 512], F32)
    recip_d = nc.dram_tensor("recip_scratch", [NH // 2, NQ, 2, 512], F32)

    with tile.TileContext(nc) as tc, ExitStack() as ctx:
        const = ctx.enter_context(tc.tile_pool(name="const", bufs=1))
        xp = ctx.enter_context(tc.tile_pool(name="xp", bufs=1))
        qp = ctx.enter_context(tc.tile_pool(name="qp", bufs=1))
        kp = ctx.enter_context(tc.tile_pool(name="kp", bufs=1))
        vp = ctx.enter_context(tc.tile_pool(name="vp", bufs=1))
        anp = ctx.enter_context(tc.tile_pool(name="anp", bufs=1))
        outp = ctx.enter_context(tc.tile_pool(name="outp", bufs=2))
        pwp = ctx.enter_context(tc.tile_pool(name="pwp", bufs=1))

        # ---------- constants ----------
        vb_sb = const.tile([128, C], F32)
        nc.sync.dma_start(vb_sb[:], vb_d[:])
        qb_sb = const.tile([128, 12], F32)
        nc.sync.dma_start(qb_sb[:], qb_d[:])
        w4_sb = const.tile([128, NCC], F32)
        nc.sync.dma_start(w4_sb[:], w4_d[:])
        b4_sb = const.tile([128, NCC], F32)
        nc.sync.dma_start(b4_sb[:], b4_d[:])
        pb_sb = const.tile([128, NCC], F32)
        nc.sync.dma_start(pb_sb[:], pb_d[:])
        gexp_sb = const.tile([NG, C], F32R)
        nc.sync.dma_start(gexp_sb[:], gexp_d[:])

        # ---------- load X ----------
        x_sb = [xp.tile([128, S], F32R, tag=f"x{cc}", name=f"x{cc}") for cc in range(NCC)]
        for cc in range(NCC):
            nc.sync.dma_start(x_sb[cc][:], x_d[128 * cc:128 * (cc + 1), :])

        # proj weights: loaded early, used at the end (DMA is idle early anyway)
        pw_sb = [pwp.tile([128, C], F32R, tag=f"pw{cc}", name=f"pw{cc}") for cc in range(NCC)]
        for cc in range(NCC):
            nc.sync.dma_start(pw_sb[cc][:], projw_d[128 * cc:128 * (cc + 1), :])

        q_sb = [qp.tile([128, S], F32R, tag=f"q{cc}", name=f"q{cc}") for cc in range(NCC)]
        k_sb = [kp.tile([128, S], F32R, tag=f"k{cc}", name=f"k{cc}") for cc in range(NCC)]
        # [64 v-channels | 1.0] per head block: the ones column turns the
        # attn@V matmul (M=65) into attn@V plus the softmax denominator row.
        vT_sb = [vp.tile([128, 65 * NH], F32R, tag=f"v{sc}", name=f"v{sc}")
                 for sc in range(NSC)]
        an_sb = [anp.tile([128, S], F32R, tag=f"an{cc}", name=f"an{cc}") for cc in range(NCC)]

        with ExitStack() as phase1:
            gnp = phase1.enter_context(tc.tile_pool(name="gnp", bufs=1))
            xsqp = phase1.enter_context(tc.tile_pool(name="xsqp", bufs=2))
            xnp = phase1.enter_context(tc.tile_pool(name="xnp", bufs=1))
            wqp = phase1.enter_context(tc.tile_pool(name="wqp", bufs=1))
            gn_es = ExitStack()
            gn_ps = gn_es.enter_context(
                tc.tile_pool(name="gn_ps", bufs=1, space="PSUM"))
            small_ps = gn_es.enter_context(
                tc.tile_pool(name="small_ps", bufs=2, space="PSUM"))

            gsum_sb = gnp.tile([C // NCC, NG * NCC], F32R)  # [128, 32*4] chunks
            for cc in range(NCC):
                nc.sync.dma_start(
                    gsum_sb[:, NG * cc:NG * (cc + 1)],
                    gsum_d[128 * cc:128 * (cc + 1), :])

            qkvw_sb = [wqp.tile([128, 3 * C], F32R, tag=f"w{cc}", name=f"w{cc}")
                       for cc in range(NCC)]
            for cc in range(NCC):
                nc.sync.dma_start(qkvw_sb[cc][:],
                                  qkvw_d[128 * cc:128 * (cc + 1), :])

            # ---------- GroupNorm stats ----------
            ps_x = gn_ps.tile([NG, S], F32, tag="ps_x")     # 2 banks
            ps_xsq = gn_ps.tile([NG, S], F32, tag="ps_xsq")  # 2 banks
            for cc in range(NCC):
                xsq = xsqp.tile([128, S], F32R)
                nc.vector.tensor_tensor(
                    out=xsq[:], in0=x_sb[cc][:].bitcast(F32),
                    in1=x_sb[cc][:].bitcast(F32), op=mybir.AluOpType.mult)
                lhsG = gsum_sb[:, NG * cc:NG * (cc + 1)]
                for qn in range(NQ):
                    nc.tensor.matmul(
                        ps_x[:, 512 * qn:512 * (qn + 1)], lhsG,
                        x_sb[cc][:, 512 * qn:512 * (qn + 1)],
                        start=(cc == 0), stop=(cc == NCC - 1))
                    nc.tensor.matmul(
                        ps_xsq[:, 512 * qn:512 * (qn + 1)], lhsG,
                        xsq[:, 512 * qn:512 * (qn + 1)],
                        start=(cc == 0), stop=(cc == NCC - 1))

            s1 = gnp.tile([NG, 1], F32)
            s2 = gnp.tile([NG, 1], F32)
            nc.vector.tensor_reduce(out=s1[:], in_=ps_x[:],
                                    axis=mybir.AxisListType.X,
                                    op=mybir.AluOpType.add)
            nc.vector.tensor_reduce(out=s2[:], in_=ps_xsq[:],
                                    axis=mybir.AxisListType.X,
                                    op=mybir.AluOpType.add)
            inv_n = 1.0 / (GS * S)
            mean_g = gnp.tile([NG, 1], F32)
            nc.vector.tensor_scalar(out=mean_g[:], in0=s1[:], scalar1=inv_n,
                                    scalar2=None, op0=mybir.AluOpType.mult)
            ex2 = gnp.tile([NG, 1], F32)
            nc.vector.tensor_scalar(out=ex2[:], in0=s2[:], scalar1=inv_n,
                                    scalar2=None, op0=mybir.AluOpType.mult)
            var_g = gnp.tile([NG, 1], F32)
            # var = E[x^2] - mean^2  (one fused op: (mean*mean) then rsub ex2)
            nc.vector.scalar_tensor_tensor(
                out=var_g[:], in0=mean_g[:], scalar=-1.0, in1=mean_g[:],
                op0=mybir.AluOpType.mult, op1=mybir.AluOpType.mult)
            nc.vector.tensor_tensor(out=var_g[:], in0=ex2[:], in1=var_g[:],
                                    op=mybir.AluOpType.add)
            # rstd = exp(-0.5 * ln(var + eps)); ln+exp share one ACT table set
            eps_sb = gnp.tile([NG, 1], F32)
            nc.vector.memset(eps_sb[:], EPS)
            lnv = gnp.tile([NG, 1], F32)
            nc.scalar.activation(out=lnv[:], in_=var_g[:], func=AF.Ln,
                                 bias=eps_sb[:], scale=1.0)
            # stats_r[:, 0] = rstd, stats_r[:, 1] = mean (N=2 matmul rhs;
            # N=1 fp32 matmuls fail the ISA's 8-byte psum-write check)
            stats_r = gnp.tile([NG, 2], F32R)
            nc.scalar.activation(out=stats_r[:, 0:1], in_=lnv[:], func=AF.Exp,
                                 bias=0.0, scale=-0.5)
            nc.vector.tensor_copy(stats_r[:, 1:2], mean_g[:])

            # per-channel rstd/mean via tiny matmuls against the group map
            rstd_c = gnp.tile([128, NCC], F32)
            mean_c = gnp.tile([128, NCC], F32)
            for cc in range(NCC):
                ps_a = small_ps.tile([128, 2], F32, tag="alpha")
                nc.tensor.matmul(ps_a[:],
                                 gexp_sb[:, 128 * cc:128 * (cc + 1)],
                                 stats_r[:], start=True, stop=True)
                nc.vector.tensor_copy(rstd_c[:, cc:cc + 1], ps_a[:, 0:1])
                nc.vector.tensor_copy(mean_c[:, cc:cc + 1], ps_a[:, 1:2])
            alpha = gnp.tile([128, NCC], F32)
            nc.vector.tensor_tensor(out=alpha[:], in0=rstd_c[:], in1=w4_sb[:],
                                    op=mybir.AluOpType.mult)
            beta = gnp.tile([128, NCC], F32)
            nc.vector.tensor_tensor(out=beta[:], in0=alpha[:], in1=mean_c[:],
                                    op=mybir.AluOpType.mult)
            nc.vector.tensor_tensor(out=beta[:], in0=b4_sb[:], in1=beta[:],
                                    op=mybir.AluOpType.subtract)

            # ---------- GN apply ----------
            xn_sb = [xnp.tile([128, S], F32R, tag=f"xn{cc}", name=f"xn{cc}")
                     for cc in range(NCC)]
            for cc in range(NCC):
                nc.vector.tensor_scalar(
                    out=xn_sb[cc][:], in0=x_sb[cc][:].bitcast(F32),
                    scalar1=alpha[:, cc:cc + 1], scalar2=beta[:, cc:cc + 1],
                    op0=mybir.AluOpType.mult, op1=mybir.AluOpType.add)

            gn_es.close()

            # ---------- V^T (pre-transposed): out[s, vch] ----------
            with ExitStack() as ph_qkv:
                qkv_ps = ph_qkv.enter_context(
                    tc.tile_pool(name="qkv_ps", bufs=2, space="PSUM"))
                for sc in range(NSC):
                    ps_v = qkv_ps.tile([128, 512], F32, tag="psv")
                    for cc in range(NCC):
                        nc.tensor.matmul(
                            ps_v[:],
                            xn_sb[cc][:, 128 * sc:128 * (sc + 1)],
                            qkvw_sb[cc][:, 1024:1536],
                            start=(cc == 0), stop=(cc == NCC - 1))
                    vT_v = vT_sb[sc][:].rearrange("p (h u) -> p h u", u=65)
                    nc.vector.tensor_tensor(
                        out=vT_v[:, :, 0:64],
                        in0=ps_v[:].rearrange("p (h u) -> p h u", u=64),
                        in1=vb_sb[:].rearrange("p (h u) -> p h u", u=64),
                        op=mybir.AluOpType.add)
                    ones_src = bass.AP(tensor=ones_d[:].tensor,
                                       offset=ones_d[:].offset,
                                       ap=[[1, 128], [0, NH], [1, 1]])
                    nc.sync.dma_start(vT_v[:, :, 64:65], ones_src)

                # ---------- Q and K, channel-major ----------
                for oc in range(NCC):
                    ps_q = qkv_ps.tile([128, S], F32, tag="psqk")
                    for cc in range(NCC):
                        for qn in range(NQ):
                            nc.tensor.matmul(
                                ps_q[:, 512 * qn:512 * (qn + 1)],
                                qkvw_sb[cc][:, 128 * oc:128 * (oc + 1)],
                                xn_sb[cc][:, 512 * qn:512 * (qn + 1)],
                                start=(cc == 0), stop=(cc == NCC - 1))
                    nc.scalar.activation(out=q_sb[oc][:], in_=ps_q[:],
                                         func=AF.Identity,
                                         bias=qb_sb[:, oc:oc + 1], scale=1.0)
                    ps_k = qkv_ps.tile([128, S], F32, tag="psqk")
                    for cc in range(NCC):
                        for qn in range(NQ):
                            nc.tensor.matmul(
                                ps_k[:, 512 * qn:512 * (qn + 1)],
                                qkvw_sb[cc][:, 512 + 128 * oc:512 + 128 * (oc + 1)],
                                xn_sb[cc][:, 512 * qn:512 * (qn + 1)],
                                start=(cc == 0), stop=(cc == NCC - 1))
                    nc.scalar.activation(out=k_sb[oc][:], in_=ps_k[:],
                                         func=AF.Identity,
                                         bias=qb_sb[:, 4 + oc:5 + oc], scale=1.0)

        # ---------- attention, one head-pair (= one qk chunk) at a time ------
        with ExitStack() as ph_att:
            expp = ph_att.enter_context(tc.tile_pool(name="expp", bufs=2))
            dscrp = ph_att.enter_context(tc.tile_pool(name="dscrp", bufs=2))
            rbp = ph_att.enter_context(tc.tile_pool(name="rbp", bufs=2))
            recp = ph_att.enter_context(tc.tile_pool(name="recp", bufs=2))
            sc_ps = ph_att.enter_context(
                tc.tile_pool(name="sc_ps", bufs=2, space="PSUM"))
            av_ps = ph_att.enter_context(
                tc.tile_pool(name="av_ps", bufs=2, space="PSUM"))

            for p in range(NH // 2):
                hA, hB = 2 * p, 2 * p + 1
                for qn in range(NQ):
                    exp_t = expp.tile([128, NSC, S], F32R, tag="exp")
                    ps_avA = av_ps.tile([65, 512], F32, tag="avA")
                    ps_avB = av_ps.tile([65, 512], F32, tag="avB")
                    for kc in range(NSC):
                        # scores^T chunk for both heads (row-tiled pair)
                        ps_s = sc_ps.tile([128, S], F32, tag="sc")
                        nc.tensor.matmul(
                            ps_s[:, 0:512],
                            k_sb[p][0:64, 128 * kc:128 * (kc + 1)],
                            q_sb[p][0:64, 512 * qn:512 * (qn + 1)],
                            start=True, stop=True, tile_position=(0, 0))
                        nc.tensor.matmul(
                            ps_s[:, 512:1024],
                            k_sb[p][64:128, 128 * kc:128 * (kc + 1)],
                            q_sb[p][64:128, 512 * qn:512 * (qn + 1)],
                            start=True, stop=True, tile_position=(64, 0))
                        # exp of both heads' chunk in one ACT pass (scale fused)
                        nc.scalar.activation(out=exp_t[:, kc, :], in_=ps_s[:],
                                             func=AF.Exp, bias=0.0, scale=SCALE)
                        # attn @ [V | 1] per head: rows 0-63 = attn@V,
                        # row 64 = softmax denominator
                        nc.tensor.matmul(
                            ps_avA[:], vT_sb[kc][:, 65 * hA:65 * (hA + 1)],
                            exp_t[:, kc, 0:512],
                            start=(kc == 0), stop=(kc == NSC - 1))
                        nc.tensor.matmul(
                            ps_avB[:], vT_sb[kc][:, 65 * hB:65 * (hB + 1)],
                            exp_t[:, kc, 512:1024],
                            start=(kc == 0), stop=(kc == NSC - 1))

                    # denominators: evict, reciprocal on 128 partitions, bcast
                    dscrA = dscrp.tile([1, 512], F32, tag="dscrA")
                    dscrB = dscrp.tile([1, 512], F32, tag="dscrB")
                    nc.vector.tensor_copy(dscrA[:], ps_avA[64:65, :])
                    nc.vector.tensor_copy(dscrB[:], ps_avB[64:65, :])
                    nc.sync.dma_start(dens_d[p, qn, 0:1], dscrA[:])
                    nc.sync.dma_start(dens_d[p, qn, 1:2], dscrB[:])
                    d128v = dens_d[p][qn].rearrange("h (x f) -> (h x) f", f=8)
                    d128 = recp.tile([128, 8], F32, tag="d128")
                    nc.sync.dma_start(d128[:], d128v)
                    r128 = recp.tile([128, 8], F32, tag="r128")
                    rscr = recp.tile([128, 8], F32, tag="rscr")
                    nc.vector.reciprocal_approx_accurate(
                        out=r128[:], in_=d128[:], scratch=rscr[:])
                    r128v = recip_d[p][qn].rearrange("h (x f) -> (h x) f", f=8)
                    nc.sync.dma_start(r128v, r128[:])
                    rb = rbp.tile([128, 512], F32, tag="rb")
                    rsrc = recip_d[p][qn]  # [2, 512]
                    rsrc_b = bass.AP(tensor=rsrc.tensor, offset=rsrc.offset,
                                     ap=[list(rsrc.ap[0]), [0, 64],
                                         list(rsrc.ap[1])])
                    nc.sync.dma_start(rb[:], rsrc_b)
                    # evict attn@V fused with the softmax normalize
                    nc.vector.tensor_tensor(
                        out=an_sb[p][0:64, 512 * qn:512 * (qn + 1)],
                        in0=ps_avA[0:64, :], in1=rb[0:64, :],
                        op=mybir.AluOpType.mult)
                    nc.vector.tensor_tensor(
                        out=an_sb[p][64:128, 512 * qn:512 * (qn + 1)],
                        in0=ps_avB[0:64, :], in1=rb[64:128, :],
                        op=mybir.AluOpType.mult)

        # ---------- proj + bias + residual ----------
        with ExitStack() as ph_proj:
            pj_ps = ph_proj.enter_context(
                tc.tile_pool(name="pj_ps", bufs=2, space="PSUM"))
            for oc in range(NCC):
                ps_o = pj_ps.tile([128, S], F32, tag="pso")
                for cc in range(NCC):
                    for qn in range(NQ):
                        nc.tensor.matmul(
                            ps_o[:, 512 * qn:512 * (qn + 1)],
                            pw_sb[cc][:, 128 * oc:128 * (oc + 1)],
                            an_sb[cc][:, 512 * qn:512 * (qn + 1)],
                            start=(cc == 0), stop=(cc == NCC - 1))
                out_t = outp.tile([128, S], F32, tag="out")
                nc.vector.scalar_tensor_tensor(
                    out=out_t[:], in0=ps_o[:], scalar=pb_sb[:, oc:oc + 1],
                    in1=x_sb[oc][:].bitcast(F32),
                    op0=mybir.AluOpType.add, op1=mybir.AluOpType.add)
                nc.sync.dma_start(y_d[128 * oc:128 * (oc + 1), :], out_t[:])

    nc.finalize()
    return nc


_NC_CACHE = None


def _get_nc():
    global _NC_CACHE
    if _NC_CACHE is None:
        _NC_CACHE = build_nc()
    return _NC_CACHE


def make_in_maps(X, norm_w, norm_b, qkv_w, qkv_b, proj_w, proj_b):
    X = np.asarray(X, dtype=np.float32)
    norm_w = np.asarray(norm_w, dtype=np.float32)
    norm_b = np.asarray(norm_b, dtype=np.float32)
    qkv_w = np.asarray(qkv_w, dtype=np.float32)
    qkv_b = np.asarray(qkv_b, dtype=np.float32)
    proj_w = np.asarray(proj_w, dtype=np.float32)
    proj_b = np.asarray(proj_b, dtype=np.float32)

    qkv_wT = np.ascontiguousarray(qkv_w.T)                    # [512, 1536]
    proj_wT = np.ascontiguousarray(proj_w.T)                  # [512, 512]
    gsum = np.zeros((C, NG), np.float32)
    gsum[np.arange(C), np.arange(C) // GS] = 1.0
    gexpT = np.ascontiguousarray(gsum.T)                      # [32, 512]
    w4 = np.ascontiguousarray(norm_w.reshape(NCC, 128).T)     # [128, 4]
    b4 = np.ascontiguousarray(norm_b.reshape(NCC, 128).T)
    qb12 = np.ascontiguousarray(qkv_b.reshape(12, 128).T)     # [128, 12]
    vb_bcast = np.ascontiguousarray(
        np.broadcast_to(qkv_b[2 * C:3 * C], (128, C)))        # [128, 512]
    pb4 = np.ascontiguousarray(proj_b.reshape(NCC, 128).T)

    shared = {
        "qkv_wT": qkv_wT, "proj_wT": proj_wT, "gsum": gsum, "gexpT": gexpT,
        "norm_w4": w4, "norm_b4": b4, "qkv_b12": qb12, "vb_bcast": vb_bcast,
        "proj_b4": pb4, "ones128": np.ones((128, 1), np.float32),
    }
    in_maps = []
    for b in range(B):
        m = dict(shared)
        m["x"] = np.ascontiguousarray(X[b].reshape(C, S))
        in_maps.append(m)
    return in_maps


def kernel(X, norm_w, norm_b, qkv_w, qkv_b, proj_w, proj_b):
    nc = _get_nc()
    in_maps = make_in_maps(X, norm_w, norm_b, qkv_w, qkv_b, proj_w, proj_b)
    res = run_bass_kernel_spmd(nc, in_maps, core_ids=list(range(B)))
    out = np.stack([res.results[b]["y"].reshape(C, H, W) for b in range(B)])
    return out.astype(np.float32)
